# revision 4
# baseline (speedup 1.0000x reference)
"""Windowed local self-attention (CrossAttention module with the context-
overwrite bug faithfully reproduced) on 8 Trainium2 NeuronCores.

Full-input contract: kernel(**inputs) takes the unsharded tensors and
returns the full (4, 4096, 1024) output. Internally the 64 independent
windows of 256 tokens are data-parallel sharded 8-per-core; the four
projection weights are broadcast to every core. No collectives needed.

Per-core pipeline (window = 256 tokens, H=16 heads, DH=64):
  X  --PE transpose-->  XT [d, i]
  qT = Wq.T @ X.T   (lhsT=Wq tiles,  rhs=XT)          [o, i]
  kT = Wk.T @ X.T                                      [o, i]
  v  = X @ Wv       (lhsT=XT tiles,  rhs=Wv)           [j, o]
  per head h:
    simT = kT_h.T-free @ qT_h   -> [j, i] in PSUM     (j on partitions)
    es   = exp(0.125 * simT)    (ACT, PSUM->SBUF)
    S    = ones[j,64].T @ es    -> [64, i] broadcast row-sums (PE)
    rS   = 1/S                  (DVE reciprocal)
    o2u  = v_h.T-free @ es      -> [d, i] in PSUM      (AV matmul)
    o2T  = o2u * rS             (DVE, writes stacked [o, i] SBUF)
  Y = o2T.T @ Wo + ones ox bo   (lhsT=o2T tiles, rhs=Wo; bias as K=1 matmul)
All matmul operands are bitcast to float32r: full fp32 bits, 1 cycle/row
on the PE at moving free-dim >= 256 (vs 4 cycles/row for plain float32).
"""

import numpy as np

import concourse.bass as bass
import concourse.mybir as mybir
import concourse.tile as tile
from concourse import bacc, bass_utils
from concourse.bass_interp import get_hw_module
from concourse.masks import make_identity

H = 16
DH = 64
WIN = 256
D = 1024
B = 4
N = 4096
N_CORES = 8
N_WIN_TOTAL = B * N // WIN          # 64
N_WIN = N_WIN_TOTAL // N_CORES      # 8 windows per core
TOK = N_WIN * WIN                   # 2048 token rows per core
SCALE = DH ** -0.5

F32 = mybir.dt.float32
F32R = mybir.dt.float32r


def _r(ap):
    return ap.bitcast(F32R)


def _body(tc, xq, wq, wk, wv, wo, bo, out, n_win):
    nc = tc.nc
    from contextlib import ExitStack

    with ExitStack() as ctx:
        singles = ctx.enter_context(tc.tile_pool(name="singles", bufs=1))
        xpool = ctx.enter_context(tc.tile_pool(name="xpool", bufs=3))
        acts = ctx.enter_context(tc.tile_pool(name="acts", bufs=1))
        heads = ctx.enter_context(tc.tile_pool(name="heads", bufs=4))
        ypool = ctx.enter_context(tc.tile_pool(name="ypool", bufs=2))
        psum = ctx.enter_context(tc.tile_pool(name="psum", bufs=7, space="PSUM"))

        # ---- constants / weights (resident all kernel) ----
        ident = singles.tile([128, 128], F32)
        make_identity(nc, ident[:])
        ones_f = singles.tile([128, 128], F32)
        nc.gpsimd.memset(ones_f[:], 1.0)
        ones_r = singles.tile([128, 128], F32R)
        nc.vector.tensor_copy(ones_r[:], ones_f[:])
        ones64 = ones_r
        onesrow = ones_r
        bo_sb = singles.tile([1, D], F32R)
        nc.sync.dma_start(bo_sb[:], bo[None, :])

        # first window's X before the big weight DMAs so transposes start early
        x_first = [xpool.tile([128, D], F32, tag="x", name=f"x0_{i}") for i in range(2)]
        for tt in range(2):
            nc.sync.dma_start(x_first[tt][:], xq[tt * 128:(tt + 1) * 128, :])

        wsb = {}
        for name, w in (("wq", wq), ("wk", wk), ("wv", wv), ("wo", wo)):
            t = singles.tile([128, 8 * D], F32R, tag=name, name=f"sb_{name}")
            for kt in range(8):
                nc.sync.dma_start(
                    t[:, kt * D:(kt + 1) * D], w[kt * 128:(kt + 1) * 128, :]
                )
            wsb[name] = t

        for w in range(n_win):
            row0 = w * WIN
            # ---- load X (natural [i, d]) ----
            if w == 0:
                x_sb = x_first
            else:
                x_sb = [xpool.tile([128, D], F32, tag="x", name=f"x_{w}_{i}") for i in range(2)]
                for tt in range(2):
                    nc.sync.dma_start(
                        x_sb[tt][:], xq[row0 + tt * 128:row0 + (tt + 1) * 128, :]
                    )

            # ---- XT [128 d, 2048] : 8 d-tiles x 256 tokens ----
            xt = acts.tile([128, 8 * WIN], F32R, tag="xt")
            for dt_ in range(8):
                for tt in range(2):
                    pt = psum.tile([128, 128], F32, tag="ps")
                    nc.tensor.transpose(
                        pt[:], x_sb[tt][:, dt_ * 128:(dt_ + 1) * 128], ident[:]
                    )
                    nc.vector.tensor_copy(
                        xt[:, dt_ * WIN + tt * 128:dt_ * WIN + tt * 128 + 128], pt[:]
                    )

            # ---- qT, kT [128, 2048] : 8 o-tiles x 256 tokens ----
            proj = {}
            for pname, wname in (("qT", "wq"), ("kT", "wk")):
                dst = acts.tile([128, 8 * WIN], F32R, tag=pname, name=f"{pname}_{w}")
                wtile = wsb[wname]
                for ot in range(8):
                    pq = psum.tile([128, WIN], F32, tag="ps")
                    for kt in range(8):
                        nc.tensor.matmul(
                            pq[:],
                            _r(wtile[:, kt * D + ot * 128:kt * D + (ot + 1) * 128]),
                            _r(xt[:, kt * WIN:(kt + 1) * WIN]),
                            start=(kt == 0),
                            stop=(kt == 7),
                        )
                    nc.vector.tensor_copy(dst[:, ot * WIN:(ot + 1) * WIN], pq[:])
                proj[pname] = dst
            qT, kT = proj["qT"], proj["kT"]

            # ---- v natural [128 j, 2048] : 2 j-tiles x 1024 o ----
            v_sb = acts.tile([128, 2 * D], F32R, tag="v")
            for jt in range(2):
                for oc in range(2):
                    pv = psum.tile([128, 512], F32, tag="ps")
                    for kt in range(8):
                        nc.tensor.matmul(
                            pv[:],
                            _r(xt[:, kt * WIN + jt * 128:kt * WIN + (jt + 1) * 128]),
                            _r(wsb["wv"][:, kt * D + oc * 512:kt * D + (oc + 1) * 512]),
                            start=(kt == 0),
                            stop=(kt == 7),
                        )
                    nc.vector.tensor_copy(
                        v_sb[:, jt * D + oc * 512:jt * D + (oc + 1) * 512], pv[:]
                    )

            # ---- attention, head by head ----
            o2T = acts.tile([128, 8 * WIN], F32R, tag="o2T")
            for h in range(H):
                prow = (h % 2) * 64
                ocol = (h // 2) * WIN
                qh = qT[prow:prow + 64, ocol:ocol + WIN]
                kh = kT[prow:prow + 64, ocol:ocol + WIN]

                es = []
                for jt in range(2):
                    ps_sim = psum.tile([128, WIN], F32, tag="ps")
                    nc.tensor.matmul(
                        ps_sim[:],
                        _r(kh[:, jt * 128:(jt + 1) * 128]),
                        _r(qh),
                        start=True,
                        stop=True,
                    )
                    e = heads.tile([128, WIN], F32R, tag="es", name=f"es_{w}_{h}_{jt}")
                    nc.scalar.activation(
                        e[:], ps_sim[:], mybir.ActivationFunctionType.Exp, scale=SCALE
                    )
                    es.append(e)

                ps_sum = psum.tile([64, WIN], F32, tag="ps")
                for jt in range(2):
                    nc.tensor.matmul(
                        ps_sum[:],
                        _r(ones64[:, 0:64]),
                        _r(es[jt][:]),
                        start=(jt == 0),
                        stop=(jt == 1),
                    )
                rs = heads.tile([64, WIN], F32, tag="rs")
                nc.vector.reciprocal(rs[:], ps_sum[:])

                ps_av = psum.tile([64, WIN], F32, tag="ps")
                for jt in range(2):
                    nc.tensor.matmul(
                        ps_av[:],
                        _r(v_sb[:, jt * D + h * DH:jt * D + (h + 1) * DH]),
                        _r(es[jt][:]),
                        start=(jt == 0),
                        stop=(jt == 1),
                    )
                nc.vector.tensor_mul(
                    o2T[prow:prow + 64, ocol:ocol + WIN], ps_av[:], rs[:]
                )

            # ---- Y = o2 @ Wo + bo   (natural [i, e]) ----
            for it in range(2):
                for ec in range(2):
                    py = psum.tile([128, 512], F32, tag="ps")
                    for kt2 in range(8):
                        nc.tensor.matmul(
                            py[:],
                            _r(o2T[:, kt2 * WIN + it * 128:kt2 * WIN + (it + 1) * 128]),
                            _r(wsb["wo"][:, kt2 * D + ec * 512:kt2 * D + (ec + 1) * 512]),
                            start=(kt2 == 0),
                            stop=False,
                        )
                    nc.tensor.matmul(
                        py[:],
                        _r(onesrow[0:1, 0:128]),
                        _r(bo_sb[:, ec * 512:(ec + 1) * 512]),
                        start=False,
                        stop=True,
                    )
                    y_sb = ypool.tile([128, 512], F32, tag="y")
                    nc.vector.tensor_copy(y_sb[:], py[:])
                    nc.sync.dma_start(
                        out[row0 + it * 128:row0 + (it + 1) * 128,
                            ec * 512:(ec + 1) * 512],
                        y_sb[:],
                    )


_CACHE = {}


def _build(n_win=N_WIN):
    key = n_win
    if key in _CACHE:
        return _CACHE[key]
    tok = n_win * WIN
    nc = bacc.Bacc(
        "TRN2", target_bir_lowering=False, debug=False, num_devices=N_CORES
    )
    xq = nc.dram_tensor("xq", [tok, D], F32, kind="ExternalInput").ap()
    wq = nc.dram_tensor("Wq", [D, D], F32R, kind="ExternalInput").ap()
    wk = nc.dram_tensor("Wk", [D, D], F32R, kind="ExternalInput").ap()
    wv = nc.dram_tensor("Wv", [D, D], F32R, kind="ExternalInput").ap()
    wo = nc.dram_tensor("Wo", [D, D], F32R, kind="ExternalInput").ap()
    bo = nc.dram_tensor("bo", [D], F32R, kind="ExternalInput").ap()
    out = nc.dram_tensor("out", [tok, D], F32, kind="ExternalOutput").ap()
    with tile.TileContext(nc) as tc:
        _body(tc, xq, wq, wk, wv, wo, bo, out, n_win)
    nc.compile()
    nc.m = get_hw_module(nc.m)
    _CACHE[key] = nc
    return nc


def run(query, Wq, Wk, Wv, Wo, bo, n_win=N_WIN, **spmd_kwargs):
    nc = _build(n_win)
    tok = n_win * WIN
    q2 = np.ascontiguousarray(np.asarray(query, dtype=np.float32).reshape(-1, D))
    weights = {
        "Wq": np.ascontiguousarray(np.asarray(Wq, np.float32)),
        "Wk": np.ascontiguousarray(np.asarray(Wk, np.float32)),
        "Wv": np.ascontiguousarray(np.asarray(Wv, np.float32)),
        "Wo": np.ascontiguousarray(np.asarray(Wo, np.float32)),
        "bo": np.ascontiguousarray(np.asarray(bo, np.float32)),
    }
    in_maps = []
    for c in range(N_CORES):
        m = {"xq": q2[c * TOK:c * TOK + tok]}
        m.update(weights)
        in_maps.append(m)
    res = bass_utils.run_bass_kernel_spmd(
        nc, in_maps, core_ids=list(range(N_CORES)), **spmd_kwargs
    )
    outs = [res.results[c]["out"] for c in range(N_CORES)]
    return outs, res


def kernel(query, context, Wq, Wk, Wv, Wo, bo):
    outs, _ = run(query, Wq, Wk, Wv, Wo, bo)
    y = np.concatenate(outs, axis=0).reshape(B, N, D)
    return y.astype(np.float32)
